# revision 30
# baseline (speedup 1.0000x reference)
"""CondConv (MoE routed conv) Trainium2 Bass kernel.

Strategy (8 NeuronCores, data-parallel over batch, 2 samples/core):
  - All conv data (x, expert slabs, combined weights cw) in bf16.
  - Routing (GAP + linear + sigmoid, 16x8 outputs, ~0.001% of FLOPs) is
    computed on the host; r broadcast rows and diag(r_e) tiles ship as
    inputs. This removes the x(s1)-arrival -> GAP -> logits -> r1 chain
    that otherwise gates every s1 weight combine.
  - PE does conv + three startup/deadline-critical combines via the
    diagonal trick: (s0,ot0,it0) hidden in the DMA wait window,
    (s1,ot0,it0) filling the phase-A->phase-B gap, (s0,ot1,it0) between
    conv phases. ACT copies PSUM chunks into bf16 cw tiles.
  - DVE runs the other five combines as scalar_tensor_tensor chains,
    paced by slab arrival; every deadline has >5us slack.
  - ACT: BN+SiLU epilogues + output DMAs on the scalar ring (in-order
    with the epilogue ACT, no cross-engine handoff).
  - DMA order: x(s0), ot0/it0 slabs, ot0/it1, x(s1), ot1 slabs; diag and
    small tensors on the gpsimd ring so x starts immediately. Expert
    slabs packed in pairs (4608B DMA rows).
  - Phase-A/B tap-outer in khkw order over 5 PSUM groups; stationary
    reuse across blocks. Junk bf16 matmuls on a memset tile bridge PE
    idle at the start to hold the HAM clock-gate at full rate.
"""

import sys

sys.path.insert(0, "/opt/trn_rl_repo")

import numpy as np
import ml_dtypes

import concourse.bass as bass  # noqa: F401
import concourse.mybir as mybir
import concourse.tile as tile
from concourse import bacc
from concourse.bass_utils import run_bass_kernel_spmd

F32 = mybir.dt.float32
BF16 = mybir.dt.bfloat16
AF = mybir.ActivationFunctionType
ALU = mybir.AluOpType

B, CIN, H, W = 16, 256, 56, 56
E, COUT, KS = 8, 256, 3
NCORES = 8
SPC = B // NCORES
IT = CIN // 128
OT = COUT // 128
KHKW = KS * KS
HB = 8  # 7 h-blocks of 8 rows, N = 448
WP = W + 2
PIX = H * W
BN_EPS = 1e-5
SLAB = KHKW * 128  # 1152
CHUNK = 384
NCH = SLAB // CHUNK
NPA = 5  # phase-A open PSUM groups (= psc pool size)

_PROGRAM_CACHE = {}


def _build_program():
    nc = bacc.Bacc("TRN2", target_bir_lowering=False, debug=False)

    x_d = nc.dram_tensor("x", [SPC, IT, 128, H, WP], BF16, kind="ExternalInput")
    # expert slabs packed in pairs: 4608B per-partition DMA rows
    wt_d = nc.dram_tensor(
        "wt", [OT, IT, E // 2, 128, 2 * SLAB], BF16, kind="ExternalInput"
    )
    diag_d = nc.dram_tensor("diag", [SPC, 128, E * 128], BF16, kind="ExternalInput")
    rbc_d = nc.dram_tensor("rbc", [SPC, 128, E], F32, kind="ExternalInput")
    bns_d = nc.dram_tensor("bns", [OT, 128, 1], F32, kind="ExternalInput")
    bnb_d = nc.dram_tensor("bnb", [OT, 128, 1], F32, kind="ExternalInput")
    y_d = nc.dram_tensor("y", [SPC, OT, 128, H, W], F32, kind="ExternalOutput")

    with tile.TileContext(nc) as tc:
        with (
            tc.tile_pool(name="xp", bufs=1) as xp,
            tc.tile_pool(name="cwp", bufs=1) as cwp,
            tc.tile_pool(name="wtp", bufs=16) as wtp,
            tc.tile_pool(name="outp", bufs=4) as outp,
            tc.tile_pool(name="smal", bufs=1) as smal,
            tc.tile_pool(name="psc", bufs=NPA, space="PSUM") as psc,
            tc.tile_pool(name="psk", bufs=2, space="PSUM") as psk,
        ):
            # ---- sync ring: x(s0), ot0 slabs (it0, it1), x(s1), ot1 ----
            x_sb = {}

            def load_x(s, its):
                for it in its:
                    t = xp.tile(
                        [128, H, WP], BF16, tag=f"x_{s}_{it}", name=f"x_{s}_{it}"
                    )
                    nc.sync.dma_start(t[:], x_d[s, it])
                    x_sb[s, it] = t

            slab_tiles = {}  # (ot, it, e) -> (pair_tile, base_col)

            def load_slabs(ot, its):
                for it in its:
                    for p in range(E // 2):
                        wt_t = wtp.tile(
                            [128, 2 * SLAB], BF16, tag="wt", name=f"wt{ot}{it}{p}"
                        )
                        nc.sync.dma_start(wt_t[:], wt_d[ot, it, p])
                        slab_tiles[ot, it, 2 * p] = (wt_t, 0)
                        slab_tiles[ot, it, 2 * p + 1] = (wt_t, SLAB)

            # phase-A needs only x(s0,it0) + it0/ot0 slabs: front = 3.2MB.
            # ot1 slabs ride ahead of x(s1): conv order is s0/ot0, s0/ot1,
            # s1/ot0, s1/ot1, so s0's ot1 combines need slabs before x(s1)
            # is needed at all.
            load_x(0, [0])
            load_slabs(0, [0])
            load_x(0, [1])
            load_slabs(0, [1])
            load_slabs(1, range(IT))
            load_x(1, range(IT))

            # ---- gpsimd ring: diag, r_bcast, bn ----
            diag_sb = {}
            for s in range(SPC):
                t = smal.tile([128, E * 128], BF16, tag=f"diag{s}", name=f"diag{s}")
                nc.gpsimd.dma_start(t[:], diag_d[s])
                diag_sb[s] = t
            r_bcast = {}
            for s in range(SPC):
                t = smal.tile([128, E], F32, tag=f"rbc{s}", name=f"rbc{s}")
                nc.gpsimd.dma_start(t[:], rbc_d[s])
                r_bcast[s] = t
            bns_sb, bnb_sb = [], []
            for ot in range(OT):
                ts_ = smal.tile([128, 1], F32, tag=f"bns{ot}", name=f"bns{ot}")
                nc.gpsimd.dma_start(ts_[:], bns_d[ot])
                bns_sb.append(ts_)
                tb_ = smal.tile([128, 1], F32, tag=f"bnb{ot}", name=f"bnb{ot}")
                nc.gpsimd.dma_start(tb_[:], bnb_d[ot])
                bnb_sb.append(tb_)

            # junk tile for warmup (memset: no DMA dependency)
            jnk_sb = smal.tile([128, CHUNK], BF16, tag="jnk")
            nc.vector.memset(jnk_sb[:], 0.5)

            def warmup(n):
                # junk bf16 matmuls keep the PE HAM clock-gate at K=8/8
                for _ in range(n):
                    wps = psk.tile([128, CHUNK], F32, tag="kps", name="wps")
                    nc.tensor.matmul(
                        wps[:], jnk_sb[:, 0:128], jnk_sb[:], start=True, stop=True
                    )

            cw_r = {
                (s, it, ot): cwp.tile(
                    [128, SLAB], BF16,
                    tag=f"cwr_{s}_{it}_{ot}", name=f"cwr_{s}_{it}_{ot}",
                )
                for s in range(SPC)
                for it in range(IT)
                for ot in range(OT)
            }

            def combine_pe(ot, s, its):
                # PE diag trick, chunk-outer/expert-inner; ACT copies PSUM->cw
                for it in its:
                    for c in range(NCH):
                        kps = psk.tile([128, CHUNK], F32, tag="kps", name="kps")
                        for e in range(E):
                            wt_t, base = slab_tiles[ot, it, e]
                            nc.tensor.matmul(
                                kps[:],
                                diag_sb[s][:, e * 128 : (e + 1) * 128],
                                wt_t[:, base + c * CHUNK : base + (c + 1) * CHUNK],
                                start=(e == 0),
                                stop=(e == E - 1),
                            )
                        nc.scalar.activation(
                            cw_r[s, it, ot][:, c * CHUNK : (c + 1) * CHUNK],
                            kps[:],
                            AF.Copy,
                        )

            def combine_eng(eng, ot, s, its):
                # multiply-accumulate chains, expert-major (slab-arrival paced)
                for it in its:
                    dst = cw_r[s, it, ot]
                    for e in range(E):
                        wt_t, base = slab_tiles[ot, it, e]
                        src = wt_t[:, base : base + SLAB]
                        sc = r_bcast[s][:, e : e + 1]
                        if e == 0:
                            eng.tensor_scalar_mul(dst[:], src, sc)
                        else:
                            eng.scalar_tensor_tensor(
                                dst[:], src, sc, dst[:], ALU.mult, ALU.add
                            )

            hblocks = [(h0, min(HB, H - h0)) for h0 in range(0, H, HB)]
            # khkw (= chunk-major) tap order
            taps = [(dh, dw) for dh in (-1, 0, 1) for dw in (-1, 0, 1)]

            def block_total(h0, nh):
                return IT * sum(
                    1 for dh, dw in taps if min(h0 + nh, H - dh) > max(h0, -dh)
                )

            def emit_tap(ot, s, it, dh, dw, h0, nh, ps_t, n_mm, total):
                khkw = (dh + 1) * 3 + (dw + 1)
                ho_s = max(h0, -dh)
                ho_e = min(h0 + nh, H - dh)
                if ho_e <= ho_s:
                    return n_mm
                nhh = ho_e - ho_s
                hi_s = ho_s + dh
                off = khkw * 128
                lhsT = cw_r[s, it, ot][:, off : off + 128]
                rhs = x_sb[s, it][:, hi_s : hi_s + nhh, 1 + dw : 1 + dw + W]
                out = ps_t[:, ho_s - h0 : ho_s - h0 + nhh, 0:W]
                nc.tensor.matmul(
                    out, lhsT, rhs,
                    start=(n_mm == 0), stop=(n_mm == total - 1),
                )
                return n_mm + 1

            def conv_taps_outer(ot, s, its, blocks, state):
                # tap-outer (khkw order): stationary reuse across blocks
                for it in its:
                    for dh, dw in taps:
                        for bi in blocks:
                            h0, nh, ps_t = state[bi][:3]
                            state[bi][3] = emit_tap(
                                ot, s, it, dh, dw, h0, nh, ps_t,
                                state[bi][3], state[bi][4],
                            )

            def conv_epilogue(ot, s, h0, nh, ps_t):
                o_t = outp.tile([128, HB, W], F32, tag="out", name="o_t")
                nc.scalar.activation(
                    o_t[:, :nh, :], ps_t[:, :nh, :], AF.Silu,
                    bias=bnb_sb[ot][:], scale=bns_sb[ot][:],
                )
                nc.scalar.dma_start(y_d[s, ot, :, h0 : h0 + nh, :], o_t[:, :nh, :])

            def conv_block_taps(ot, s, h0, nh, ps_t, its, n_mm, total):
                for it in its:
                    for dh, dw in taps:
                        n_mm = emit_tap(ot, s, it, dh, dw, h0, nh, ps_t, n_mm, total)
                return n_mm

            def conv_full_block(ot, s, h0, nh):
                ps_t = psc.tile([128, HB, W], F32, tag="ps", name="ps")
                total = block_total(h0, nh)
                n_mm = conv_block_taps(ot, s, h0, nh, ps_t, range(IT), 0, total)
                assert n_mm == total
                conv_epilogue(ot, s, h0, nh, ps_t)

            # ================= emission schedule =================
            warmup(48)

            combine_pe(0, 0, [0])
            combine_eng(nc.vector, 0, 0, [1])

            # phase-A: it0 taps, tap-outer, first NPA blocks of (s0, ot0)
            state = {}
            for bi, (h0, nh) in enumerate(hblocks[:NPA]):
                ps_t = psc.tile([128, HB, W], F32, tag="ps", name="ps")
                state[bi] = [h0, nh, ps_t, 0, block_total(h0, nh)]
            conv_taps_outer(0, 0, [0], list(range(NPA)), state)

            # s1/ot0 it0 on PE fills the phase-A -> phase-B wait; DVE
            # chains run in conv-deadline order
            combine_pe(0, 1, [0])
            combine_eng(nc.vector, 1, 0, [1])
            combine_eng(nc.vector, 0, 1, [1])
            combine_eng(nc.vector, 1, 1, range(IT))

            # phase-B: close phase-A blocks with it1 taps
            conv_taps_outer(0, 0, [1], list(range(NPA)), state)
            for bi in range(NPA):
                h0, nh, ps_t, n_mm, total = state[bi]
                assert n_mm == total
                conv_epilogue(0, 0, h0, nh, ps_t)

            # remaining s0/ot0 blocks; then s0/ot1 ((s0,ot1,it0) PE combine
            # first), s1/ot0, s1/ot1
            for h0, nh in hblocks[NPA:]:
                conv_full_block(0, 0, h0, nh)
            combine_pe(1, 0, [0])
            for h0, nh in hblocks:
                conv_full_block(1, 0, h0, nh)
            for h0, nh in hblocks:
                conv_full_block(0, 1, h0, nh)
            for h0, nh in hblocks:
                conv_full_block(1, 1, h0, nh)

    nc.compile()
    return nc


def _get_program():
    if "nc" not in _PROGRAM_CACHE:
        _PROGRAM_CACHE["nc"] = _build_program()
    return _PROGRAM_CACHE["nc"]


def kernel(x, routing_w, routing_b, kernel_weights, bn_gamma, bn_beta, bn_mean, bn_var,
           _trace=False, _trace_kwargs=None):
    x = np.asarray(x, dtype=np.float32)
    routing_w = np.asarray(routing_w, dtype=np.float32)
    routing_b = np.asarray(routing_b, dtype=np.float32)
    kernel_weights = np.asarray(kernel_weights, dtype=np.float32)
    bn_gamma = np.asarray(bn_gamma, dtype=np.float32)
    bn_beta = np.asarray(bn_beta, dtype=np.float32)
    bn_mean = np.asarray(bn_mean, dtype=np.float32)
    bn_var = np.asarray(bn_var, dtype=np.float32)

    bf16 = ml_dtypes.bfloat16
    # host routing: r = sigmoid(GAP(x) @ routing_w.T + routing_b)  (16x8)
    pooled = x.mean(axis=(2, 3))
    logits = pooled @ routing_w.T + routing_b
    r = 1.0 / (1.0 + np.exp(-logits))  # (B, E) fp64 -> fp32
    r = r.astype(np.float32)
    eye = np.eye(128, dtype=np.float32)
    # diag[s, :, e*128:(e+1)*128] = r[s, e] * I
    diag_all = (r[:, :, None, None] * eye[None, None]).transpose(0, 2, 1, 3)
    diag_host_all = np.ascontiguousarray(diag_all).reshape(B, 128, E * 128).astype(bf16)
    rbc_host_all = np.broadcast_to(r[:, None, :], (B, 128, E)).astype(np.float32)

    # wt[ot, it, pair, i, half*SLAB + khkw*128 + o_in]
    # from kernel_weights[e, o, i, kh, kw]; experts packed in pairs
    kw7 = kernel_weights.reshape(E, OT, 128, IT, 128, KS, KS)
    wt_e = np.ascontiguousarray(kw7.transpose(1, 3, 0, 4, 5, 6, 2)).reshape(
        OT, IT, E, 128, SLAB
    )
    wt_host = np.ascontiguousarray(
        wt_e.reshape(OT, IT, E // 2, 2, 128, SLAB).transpose(0, 1, 2, 4, 3, 5)
    ).reshape(OT, IT, E // 2, 128, 2 * SLAB).astype(bf16)
    inv = bn_gamma / np.sqrt(bn_var + BN_EPS)
    bns_host = np.ascontiguousarray(inv).reshape(OT, 128, 1)
    bnb_host = np.ascontiguousarray(bn_beta - bn_mean * inv).reshape(OT, 128, 1)

    x_pad = np.zeros((B, CIN, H, WP), dtype=np.float32)
    x_pad[:, :, :, 1 : 1 + W] = x
    x_pad = x_pad.astype(bf16)
    in_maps = []
    for g in range(NCORES):
        xg = np.ascontiguousarray(
            x_pad[g * SPC : (g + 1) * SPC].reshape(SPC, IT, 128, H, WP)
        )
        in_maps.append(
            {
                "x": xg,
                "wt": wt_host,
                "diag": np.ascontiguousarray(diag_host_all[g * SPC : (g + 1) * SPC]),
                "rbc": np.ascontiguousarray(rbc_host_all[g * SPC : (g + 1) * SPC]),
                "bns": bns_host,
                "bnb": bnb_host,
            }
        )

    nc = _get_program()
    res = run_bass_kernel_spmd(
        nc, in_maps, core_ids=list(range(NCORES)),
        trace=_trace, **(_trace_kwargs or {}),
    )
    _PROGRAM_CACHE["last_result"] = res

    out = np.empty((B, COUT, H, W), dtype=np.float32)
    for g in range(NCORES):
        yg = res.results[g]["y"]
        out[g * SPC : (g + 1) * SPC] = yg.reshape(SPC, COUT, H, W)
    return out


# revision 35
# speedup vs baseline: 1.0049x; 1.0049x over previous
"""CondConv (MoE routed conv) Trainium2 Bass kernel.

Strategy (8 NeuronCores, data-parallel over batch, 2 samples/core):
  - All conv data (x, expert slabs, combined weights cw) in bf16.
  - Routing (GAP + linear + sigmoid, 16x8 outputs, ~0.001% of FLOPs) is
    computed on the host; r broadcast rows and diag(r_e) tiles ship as
    inputs. This removes the x(s1)-arrival -> GAP -> logits -> r1 chain
    that otherwise gates every s1 weight combine.
  - PE does conv + three startup/deadline-critical combines via the
    diagonal trick: (s0,ot0,it0) hidden in the DMA wait window,
    (s1,ot0,it0) filling the phase-A->phase-B gap, (s0,ot1,it0) between
    conv phases. ACT copies PSUM chunks into bf16 cw tiles.
  - DVE runs the other five combines as scalar_tensor_tensor chains,
    paced by slab arrival; every deadline has >5us slack.
  - ACT: BN+SiLU epilogues + output DMAs on the scalar ring (in-order
    with the epilogue ACT, no cross-engine handoff).
  - DMA order: x(s0), ot0/it0 slabs, ot0/it1, x(s1), ot1 slabs; diag and
    small tensors on the gpsimd ring so x starts immediately. Expert
    slabs packed in pairs (4608B DMA rows).
  - Phase-A/B tap-outer in khkw order over 5 PSUM groups; stationary
    reuse across blocks. Junk bf16 matmuls on a memset tile bridge PE
    idle at the start to hold the HAM clock-gate at full rate.
"""

import sys

sys.path.insert(0, "/opt/trn_rl_repo")

import numpy as np
import ml_dtypes

import concourse.bass as bass  # noqa: F401
import concourse.mybir as mybir
import concourse.tile as tile
from concourse import bacc
from concourse.bass_utils import run_bass_kernel_spmd

F32 = mybir.dt.float32
BF16 = mybir.dt.bfloat16
AF = mybir.ActivationFunctionType
ALU = mybir.AluOpType

B, CIN, H, W = 16, 256, 56, 56
E, COUT, KS = 8, 256, 3
NCORES = 8
SPC = B // NCORES
IT = CIN // 128
OT = COUT // 128
KHKW = KS * KS
HB = 8  # 7 h-blocks of 8 rows, N = 448
WP = W + 2
PIX = H * W
BN_EPS = 1e-5
SLAB = KHKW * 128  # 1152
CHUNK = 384
NCH = SLAB // CHUNK
NPA = 5  # phase-A open PSUM groups (= psc pool size)

_PROGRAM_CACHE = {}


def _build_program():
    nc = bacc.Bacc("TRN2", target_bir_lowering=False, debug=False)

    x_d = nc.dram_tensor("x", [SPC, IT, 128, H, WP], BF16, kind="ExternalInput")
    # expert slabs packed in pairs: 4608B per-partition DMA rows
    wt_d = nc.dram_tensor(
        "wt", [OT, IT, E // 2, 128, 2 * SLAB], BF16, kind="ExternalInput"
    )
    diag_d = nc.dram_tensor("diag", [SPC, 128, E * 128], BF16, kind="ExternalInput")
    rbc_d = nc.dram_tensor("rbc", [SPC, 128, E], F32, kind="ExternalInput")
    bns_d = nc.dram_tensor("bns", [OT, 128, 1], F32, kind="ExternalInput")
    bnb_d = nc.dram_tensor("bnb", [OT, 128, 1], F32, kind="ExternalInput")
    y_d = nc.dram_tensor("y", [SPC, OT, 128, H, W], BF16, kind="ExternalOutput")

    with tile.TileContext(nc) as tc:
        with (
            tc.tile_pool(name="xp", bufs=1) as xp,
            tc.tile_pool(name="cwp", bufs=1) as cwp,
            tc.tile_pool(name="wtp", bufs=16) as wtp,
            tc.tile_pool(name="outp", bufs=4) as outp,
            tc.tile_pool(name="smal", bufs=1) as smal,
            tc.tile_pool(name="psc", bufs=NPA, space="PSUM") as psc,
            tc.tile_pool(name="psk", bufs=2, space="PSUM") as psk,
        ):
            # ---- sync ring: x(s0), ot0 slabs (it0, it1), x(s1), ot1 ----
            x_sb = {}

            def load_x(s, its):
                for it in its:
                    t = xp.tile(
                        [128, H, WP], BF16, tag=f"x_{s}_{it}", name=f"x_{s}_{it}"
                    )
                    nc.sync.dma_start(t[:], x_d[s, it])
                    x_sb[s, it] = t

            slab_tiles = {}  # (ot, it, e) -> (pair_tile, base_col)

            def load_slabs(ot, its):
                for it in its:
                    for p in range(E // 2):
                        wt_t = wtp.tile(
                            [128, 2 * SLAB], BF16, tag="wt", name=f"wt{ot}{it}{p}"
                        )
                        nc.sync.dma_start(wt_t[:], wt_d[ot, it, p])
                        slab_tiles[ot, it, 2 * p] = (wt_t, 0)
                        slab_tiles[ot, it, 2 * p + 1] = (wt_t, SLAB)

            # phase-A needs only x(s0,it0) + it0/ot0 slabs: front = 3.2MB.
            # ot1 slabs ride ahead of x(s1): conv order is s0/ot0, s0/ot1,
            # s1/ot0, s1/ot1, so s0's ot1 combines need slabs before x(s1)
            # is needed at all.
            load_x(0, [0])
            load_slabs(0, [0])
            load_x(0, [1])
            load_slabs(0, [1])
            load_slabs(1, range(IT))
            load_x(1, range(IT))

            # ---- gpsimd ring: diag, r_bcast, bn ----
            diag_sb = {}
            for s in range(SPC):
                t = smal.tile([128, E * 128], BF16, tag=f"diag{s}", name=f"diag{s}")
                nc.gpsimd.dma_start(t[:], diag_d[s])
                diag_sb[s] = t
            r_bcast = {}
            for s in range(SPC):
                t = smal.tile([128, E], F32, tag=f"rbc{s}", name=f"rbc{s}")
                nc.gpsimd.dma_start(t[:], rbc_d[s])
                r_bcast[s] = t
            bns_sb, bnb_sb = [], []
            for ot in range(OT):
                ts_ = smal.tile([128, 1], F32, tag=f"bns{ot}", name=f"bns{ot}")
                nc.gpsimd.dma_start(ts_[:], bns_d[ot])
                bns_sb.append(ts_)
                tb_ = smal.tile([128, 1], F32, tag=f"bnb{ot}", name=f"bnb{ot}")
                nc.gpsimd.dma_start(tb_[:], bnb_d[ot])
                bnb_sb.append(tb_)

            # junk tile for warmup (memset: no DMA dependency)
            jnk_sb = smal.tile([128, CHUNK], BF16, tag="jnk")
            nc.vector.memset(jnk_sb[:], 0.5)

            def warmup(n):
                # junk bf16 matmuls keep the PE HAM clock-gate at K=8/8
                for _ in range(n):
                    wps = psk.tile([128, CHUNK], F32, tag="kps", name="wps")
                    nc.tensor.matmul(
                        wps[:], jnk_sb[:, 0:128], jnk_sb[:], start=True, stop=True
                    )

            cw_r = {
                (s, it, ot): cwp.tile(
                    [128, SLAB], BF16,
                    tag=f"cwr_{s}_{it}_{ot}", name=f"cwr_{s}_{it}_{ot}",
                )
                for s in range(SPC)
                for it in range(IT)
                for ot in range(OT)
            }

            def combine_pe(ot, s, its):
                # PE diag trick, chunk-outer/expert-inner; ACT copies PSUM->cw
                for it in its:
                    for c in range(NCH):
                        kps = psk.tile([128, CHUNK], F32, tag="kps", name="kps")
                        for e in range(E):
                            wt_t, base = slab_tiles[ot, it, e]
                            nc.tensor.matmul(
                                kps[:],
                                diag_sb[s][:, e * 128 : (e + 1) * 128],
                                wt_t[:, base + c * CHUNK : base + (c + 1) * CHUNK],
                                start=(e == 0),
                                stop=(e == E - 1),
                            )
                        nc.scalar.activation(
                            cw_r[s, it, ot][:, c * CHUNK : (c + 1) * CHUNK],
                            kps[:],
                            AF.Copy,
                        )

            def combine_eng(eng, ot, s, its, dep=None):
                # multiply-accumulate chains, expert-major (slab-arrival
                # paced). `dep`: bypass-read a prior chain's output on the
                # first op so the tile scheduler cannot interleave this
                # chain with the (deadline-critical) previous one.
                for it in its:
                    dst = cw_r[s, it, ot]
                    for e in range(E):
                        wt_t, base = slab_tiles[ot, it, e]
                        src = wt_t[:, base : base + SLAB]
                        sc = r_bcast[s][:, e : e + 1]
                        if e == 0:
                            if dep is not None:
                                eng.scalar_tensor_tensor(
                                    dst[:], src, sc, dep[:], ALU.mult, ALU.bypass
                                )
                            else:
                                eng.tensor_scalar_mul(dst[:], src, sc)
                        else:
                            eng.scalar_tensor_tensor(
                                dst[:], src, sc, dst[:], ALU.mult, ALU.add
                            )
                    dep = dst
                return dep

            hblocks = [(h0, min(HB, H - h0)) for h0 in range(0, H, HB)]
            # khkw (= chunk-major) tap order
            taps = [(dh, dw) for dh in (-1, 0, 1) for dw in (-1, 0, 1)]

            def block_total(h0, nh):
                return IT * sum(
                    1 for dh, dw in taps if min(h0 + nh, H - dh) > max(h0, -dh)
                )

            def emit_tap(ot, s, it, dh, dw, h0, nh, ps_t, n_mm, total):
                khkw = (dh + 1) * 3 + (dw + 1)
                ho_s = max(h0, -dh)
                ho_e = min(h0 + nh, H - dh)
                if ho_e <= ho_s:
                    return n_mm
                nhh = ho_e - ho_s
                hi_s = ho_s + dh
                off = khkw * 128
                lhsT = cw_r[s, it, ot][:, off : off + 128]
                rhs = x_sb[s, it][:, hi_s : hi_s + nhh, 1 + dw : 1 + dw + W]
                out = ps_t[:, ho_s - h0 : ho_s - h0 + nhh, 0:W]
                nc.tensor.matmul(
                    out, lhsT, rhs,
                    start=(n_mm == 0), stop=(n_mm == total - 1),
                )
                return n_mm + 1

            def conv_taps_outer(ot, s, its, blocks, state):
                # tap-outer (khkw order): stationary reuse across blocks
                for it in its:
                    for dh, dw in taps:
                        for bi in blocks:
                            h0, nh, ps_t = state[bi][:3]
                            state[bi][3] = emit_tap(
                                ot, s, it, dh, dw, h0, nh, ps_t,
                                state[bi][3], state[bi][4],
                            )

            def conv_epilogue(ot, s, h0, nh, ps_t):
                o_t = outp.tile([128, HB, W], BF16, tag="out", name="o_t")
                nc.scalar.activation(
                    o_t[:, :nh, :], ps_t[:, :nh, :], AF.Silu,
                    bias=bnb_sb[ot][:], scale=bns_sb[ot][:],
                )
                nc.scalar.dma_start(y_d[s, ot, :, h0 : h0 + nh, :], o_t[:, :nh, :])

            def conv_block_taps(ot, s, h0, nh, ps_t, its, n_mm, total):
                for it in its:
                    for dh, dw in taps:
                        n_mm = emit_tap(ot, s, it, dh, dw, h0, nh, ps_t, n_mm, total)
                return n_mm

            def conv_full_block(ot, s, h0, nh):
                ps_t = psc.tile([128, HB, W], F32, tag="ps", name="ps")
                total = block_total(h0, nh)
                n_mm = conv_block_taps(ot, s, h0, nh, ps_t, range(IT), 0, total)
                assert n_mm == total
                conv_epilogue(ot, s, h0, nh, ps_t)

            # ================= emission schedule =================
            warmup(48)

            combine_pe(0, 0, [0])
            combine_eng(nc.vector, 0, 0, [1])

            # phase-A: it0 taps, tap-outer, first NPA blocks of (s0, ot0)
            state = {}
            for bi, (h0, nh) in enumerate(hblocks[:NPA]):
                ps_t = psc.tile([128, HB, W], F32, tag="ps", name="ps")
                state[bi] = [h0, nh, ps_t, 0, block_total(h0, nh)]
            conv_taps_outer(0, 0, [0], list(range(NPA)), state)

            # s1/ot0 it0 on PE fills the phase-A -> phase-B wait; DVE
            # chains run serialized in conv-deadline order
            combine_pe(0, 1, [0])
            dep = combine_eng(nc.vector, 1, 0, [1], dep=cw_r[0, 1, 0])
            dep = combine_eng(nc.vector, 0, 1, [1], dep=dep)
            combine_eng(nc.vector, 1, 1, range(IT), dep=dep)

            # phase-B: close phase-A blocks with it1 taps
            conv_taps_outer(0, 0, [1], list(range(NPA)), state)
            for bi in range(NPA):
                h0, nh, ps_t, n_mm, total = state[bi]
                assert n_mm == total
                conv_epilogue(0, 0, h0, nh, ps_t)

            # remaining s0/ot0 blocks; then s0/ot1 ((s0,ot1,it0) PE combine
            # first), s1/ot0, s1/ot1
            for h0, nh in hblocks[NPA:]:
                conv_full_block(0, 0, h0, nh)
            combine_pe(1, 0, [0])
            for h0, nh in hblocks:
                conv_full_block(1, 0, h0, nh)
            for h0, nh in hblocks:
                conv_full_block(0, 1, h0, nh)
            for h0, nh in hblocks:
                conv_full_block(1, 1, h0, nh)

    nc.compile()
    return nc


def _get_program():
    if "nc" not in _PROGRAM_CACHE:
        _PROGRAM_CACHE["nc"] = _build_program()
    return _PROGRAM_CACHE["nc"]


def kernel(x, routing_w, routing_b, kernel_weights, bn_gamma, bn_beta, bn_mean, bn_var,
           _trace=False, _trace_kwargs=None):
    x = np.asarray(x, dtype=np.float32)
    routing_w = np.asarray(routing_w, dtype=np.float32)
    routing_b = np.asarray(routing_b, dtype=np.float32)
    kernel_weights = np.asarray(kernel_weights, dtype=np.float32)
    bn_gamma = np.asarray(bn_gamma, dtype=np.float32)
    bn_beta = np.asarray(bn_beta, dtype=np.float32)
    bn_mean = np.asarray(bn_mean, dtype=np.float32)
    bn_var = np.asarray(bn_var, dtype=np.float32)

    bf16 = ml_dtypes.bfloat16
    # host routing: r = sigmoid(GAP(x) @ routing_w.T + routing_b)  (16x8)
    pooled = x.mean(axis=(2, 3))
    logits = pooled @ routing_w.T + routing_b
    r = 1.0 / (1.0 + np.exp(-logits))  # (B, E) fp64 -> fp32
    r = r.astype(np.float32)
    eye = np.eye(128, dtype=np.float32)
    # diag[s, :, e*128:(e+1)*128] = r[s, e] * I
    diag_all = (r[:, :, None, None] * eye[None, None]).transpose(0, 2, 1, 3)
    diag_host_all = np.ascontiguousarray(diag_all).reshape(B, 128, E * 128).astype(bf16)
    rbc_host_all = np.broadcast_to(r[:, None, :], (B, 128, E)).astype(np.float32)

    # wt[ot, it, pair, i, half*SLAB + khkw*128 + o_in]
    # from kernel_weights[e, o, i, kh, kw]; experts packed in pairs
    kw7 = kernel_weights.reshape(E, OT, 128, IT, 128, KS, KS)
    wt_e = np.ascontiguousarray(kw7.transpose(1, 3, 0, 4, 5, 6, 2)).reshape(
        OT, IT, E, 128, SLAB
    )
    wt_host = np.ascontiguousarray(
        wt_e.reshape(OT, IT, E // 2, 2, 128, SLAB).transpose(0, 1, 2, 4, 3, 5)
    ).reshape(OT, IT, E // 2, 128, 2 * SLAB).astype(bf16)
    inv = bn_gamma / np.sqrt(bn_var + BN_EPS)
    bns_host = np.ascontiguousarray(inv).reshape(OT, 128, 1)
    bnb_host = np.ascontiguousarray(bn_beta - bn_mean * inv).reshape(OT, 128, 1)

    x_pad = np.zeros((B, CIN, H, WP), dtype=np.float32)
    x_pad[:, :, :, 1 : 1 + W] = x
    x_pad = x_pad.astype(bf16)
    in_maps = []
    for g in range(NCORES):
        xg = np.ascontiguousarray(
            x_pad[g * SPC : (g + 1) * SPC].reshape(SPC, IT, 128, H, WP)
        )
        in_maps.append(
            {
                "x": xg,
                "wt": wt_host,
                "diag": np.ascontiguousarray(diag_host_all[g * SPC : (g + 1) * SPC]),
                "rbc": np.ascontiguousarray(rbc_host_all[g * SPC : (g + 1) * SPC]),
                "bns": bns_host,
                "bnb": bnb_host,
            }
        )

    nc = _get_program()
    res = run_bass_kernel_spmd(
        nc, in_maps, core_ids=list(range(NCORES)),
        trace=_trace, **(_trace_kwargs or {}),
    )
    _PROGRAM_CACHE["last_result"] = res

    out = np.empty((B, COUT, H, W), dtype=np.float32)
    for g in range(NCORES):
        yg = res.results[g]["y"].astype(np.float32)
        out[g * SPC : (g + 1) * SPC] = yg.reshape(SPC, COUT, H, W)
    return out


# revision 36
# speedup vs baseline: 1.0282x; 1.0232x over previous
"""CondConv (MoE routed conv) Trainium2 Bass kernel.

Strategy (8 NeuronCores, data-parallel over batch, 2 samples/core):
  - All conv data (x, expert slabs, combined weights cw) in bf16.
  - Routing (GAP + linear + sigmoid, 16x8 outputs, ~0.001% of FLOPs) is
    computed on the host; r broadcast rows and diag(r_e) tiles ship as
    inputs. This removes the x(s1)-arrival -> GAP -> logits -> r1 chain
    that otherwise gates every s1 weight combine.
  - PE does conv + three startup/deadline-critical combines via the
    diagonal trick: (s0,ot0,it0) hidden in the DMA wait window,
    (s1,ot0,it0) filling the phase-A->phase-B gap, (s0,ot1,it0) between
    conv phases. ACT copies PSUM chunks into bf16 cw tiles.
  - DVE runs the other five combines as scalar_tensor_tensor chains,
    paced by slab arrival; every deadline has >5us slack.
  - ACT: BN+SiLU epilogues + output DMAs on the scalar ring (in-order
    with the epilogue ACT, no cross-engine handoff).
  - DMA order: x(s0), ot0/it0 slabs, ot0/it1, x(s1), ot1 slabs; diag and
    small tensors on the gpsimd ring so x starts immediately. Expert
    slabs packed in pairs (4608B DMA rows).
  - Phase-A/B tap-outer in khkw order over 5 PSUM groups; stationary
    reuse across blocks. Junk bf16 matmuls on a memset tile bridge PE
    idle at the start to hold the HAM clock-gate at full rate.
"""

import sys

sys.path.insert(0, "/opt/trn_rl_repo")

import numpy as np
import ml_dtypes

import concourse.bass as bass  # noqa: F401
import concourse.mybir as mybir
import concourse.tile as tile
from concourse import bacc
from concourse.bass_utils import run_bass_kernel_spmd

F32 = mybir.dt.float32
BF16 = mybir.dt.bfloat16
AF = mybir.ActivationFunctionType
ALU = mybir.AluOpType

B, CIN, H, W = 16, 256, 56, 56
E, COUT, KS = 8, 256, 3
NCORES = 8
SPC = B // NCORES
IT = CIN // 128
OT = COUT // 128
KHKW = KS * KS
HB = 8  # 7 h-blocks of 8 rows, N = 448
WP = W + 2
PIX = H * W
BN_EPS = 1e-5
SLAB = KHKW * 128  # 1152
CHUNK = 384
NCH = SLAB // CHUNK
NPA = 5  # phase-A open PSUM groups (= psc pool size)

_PROGRAM_CACHE = {}


def _build_program():
    nc = bacc.Bacc("TRN2", target_bir_lowering=False, debug=False)

    x_d = nc.dram_tensor("x", [SPC, IT, 128, H, WP], BF16, kind="ExternalInput")
    # expert slabs packed in pairs: 4608B per-partition DMA rows
    wt_d = nc.dram_tensor(
        "wt", [OT, IT, E // 2, 128, 2 * SLAB], BF16, kind="ExternalInput"
    )
    diag_d = nc.dram_tensor("diag", [SPC, 128, E * 128], BF16, kind="ExternalInput")
    rbc_d = nc.dram_tensor("rbc", [SPC, 128, E], F32, kind="ExternalInput")
    bns_d = nc.dram_tensor("bns", [OT, 128, 1], F32, kind="ExternalInput")
    bnb_d = nc.dram_tensor("bnb", [OT, 128, 1], F32, kind="ExternalInput")
    y_d = nc.dram_tensor("y", [SPC, OT, 128, H, W], BF16, kind="ExternalOutput")

    with tile.TileContext(nc) as tc:
        with (
            tc.tile_pool(name="xp", bufs=1) as xp,
            tc.tile_pool(name="cwp", bufs=1) as cwp,
            tc.tile_pool(name="wtp", bufs=16) as wtp,
            tc.tile_pool(name="outp", bufs=4) as outp,
            tc.tile_pool(name="smal", bufs=1) as smal,
            tc.tile_pool(name="psc", bufs=NPA, space="PSUM") as psc,
            tc.tile_pool(name="psk", bufs=2, space="PSUM") as psk,
        ):
            # ---- sync ring: x(s0), ot0 slabs (it0, it1), x(s1), ot1 ----
            x_sb = {}

            def load_x(s, its):
                for it in its:
                    t = xp.tile(
                        [128, H, WP], BF16, tag=f"x_{s}_{it}", name=f"x_{s}_{it}"
                    )
                    nc.sync.dma_start(t[:], x_d[s, it])
                    x_sb[s, it] = t

            slab_tiles = {}  # (ot, it, e) -> (pair_tile, base_col)

            def load_slabs(ot, its):
                for it in its:
                    for p in range(E // 2):
                        wt_t = wtp.tile(
                            [128, 2 * SLAB], BF16, tag="wt", name=f"wt{ot}{it}{p}"
                        )
                        nc.sync.dma_start(wt_t[:], wt_d[ot, it, p])
                        slab_tiles[ot, it, 2 * p] = (wt_t, 0)
                        slab_tiles[ot, it, 2 * p + 1] = (wt_t, SLAB)

            # phase-A needs only x(s0,it0) + it0/ot0 slabs: front = 3.2MB.
            # ot1 slabs ride ahead of x(s1): conv order is s0/ot0, s0/ot1,
            # s1/ot0, s1/ot1, so s0's ot1 combines need slabs before x(s1)
            # is needed at all.
            load_x(0, [0])
            load_slabs(0, [0])
            load_x(0, [1])
            load_slabs(0, [1])
            load_slabs(1, range(IT))
            load_x(1, range(IT))

            # ---- gpsimd ring: diag, r_bcast, bn ----
            diag_sb = {}
            for s in range(SPC):
                t = smal.tile([128, E * 128], BF16, tag=f"diag{s}", name=f"diag{s}")
                nc.gpsimd.dma_start(t[:], diag_d[s])
                diag_sb[s] = t
            r_bcast = {}
            for s in range(SPC):
                t = smal.tile([128, E], F32, tag=f"rbc{s}", name=f"rbc{s}")
                nc.gpsimd.dma_start(t[:], rbc_d[s])
                r_bcast[s] = t
            bns_sb, bnb_sb = [], []
            for ot in range(OT):
                ts_ = smal.tile([128, 1], F32, tag=f"bns{ot}", name=f"bns{ot}")
                nc.gpsimd.dma_start(ts_[:], bns_d[ot])
                bns_sb.append(ts_)
                tb_ = smal.tile([128, 1], F32, tag=f"bnb{ot}", name=f"bnb{ot}")
                nc.gpsimd.dma_start(tb_[:], bnb_d[ot])
                bnb_sb.append(tb_)

            # junk tile for warmup (memset: no DMA dependency)
            jnk_sb = smal.tile([128, CHUNK], BF16, tag="jnk")
            nc.vector.memset(jnk_sb[:], 0.5)

            def warmup(n):
                # junk bf16 matmuls keep the PE HAM clock-gate at K=8/8
                for _ in range(n):
                    wps = psk.tile([128, CHUNK], F32, tag="kps", name="wps")
                    nc.tensor.matmul(
                        wps[:], jnk_sb[:, 0:128], jnk_sb[:], start=True, stop=True
                    )

            cw_r = {
                (s, it, ot): cwp.tile(
                    [128, SLAB], BF16,
                    tag=f"cwr_{s}_{it}_{ot}", name=f"cwr_{s}_{it}_{ot}",
                )
                for s in range(SPC)
                for it in range(IT)
                for ot in range(OT)
            }

            def combine_pe(ot, s, its):
                # PE diag trick, chunk-outer/expert-inner; ACT copies PSUM->cw
                for it in its:
                    for c in range(NCH):
                        kps = psk.tile([128, CHUNK], F32, tag="kps", name="kps")
                        for e in range(E):
                            wt_t, base = slab_tiles[ot, it, e]
                            nc.tensor.matmul(
                                kps[:],
                                diag_sb[s][:, e * 128 : (e + 1) * 128],
                                wt_t[:, base + c * CHUNK : base + (c + 1) * CHUNK],
                                start=(e == 0),
                                stop=(e == E - 1),
                            )
                        nc.scalar.activation(
                            cw_r[s, it, ot][:, c * CHUNK : (c + 1) * CHUNK],
                            kps[:],
                            AF.Copy,
                        )

            def combine_eng(eng, ot, s, its, dep=None):
                # multiply-accumulate chains, expert-major (slab-arrival
                # paced). `dep`: bypass-read a prior chain's output on the
                # first op so the tile scheduler cannot interleave this
                # chain with the (deadline-critical) previous one.
                for it in its:
                    dst = cw_r[s, it, ot]
                    for e in range(E):
                        wt_t, base = slab_tiles[ot, it, e]
                        src = wt_t[:, base : base + SLAB]
                        sc = r_bcast[s][:, e : e + 1]
                        if e == 0:
                            if dep is not None:
                                eng.scalar_tensor_tensor(
                                    dst[:], src, sc, dep[:], ALU.mult, ALU.bypass
                                )
                            else:
                                eng.tensor_scalar_mul(dst[:], src, sc)
                        else:
                            eng.scalar_tensor_tensor(
                                dst[:], src, sc, dst[:], ALU.mult, ALU.add
                            )
                    dep = dst
                return dep

            hblocks = [(h0, min(HB, H - h0)) for h0 in range(0, H, HB)]
            # khkw (= chunk-major) tap order
            taps = [(dh, dw) for dh in (-1, 0, 1) for dw in (-1, 0, 1)]

            def block_total(h0, nh):
                return IT * sum(
                    1 for dh, dw in taps if min(h0 + nh, H - dh) > max(h0, -dh)
                )

            def emit_tap(ot, s, it, dh, dw, h0, nh, ps_t, n_mm, total):
                khkw = (dh + 1) * 3 + (dw + 1)
                ho_s = max(h0, -dh)
                ho_e = min(h0 + nh, H - dh)
                if ho_e <= ho_s:
                    return n_mm
                nhh = ho_e - ho_s
                hi_s = ho_s + dh
                off = khkw * 128
                lhsT = cw_r[s, it, ot][:, off : off + 128]
                rhs = x_sb[s, it][:, hi_s : hi_s + nhh, 1 + dw : 1 + dw + W]
                out = ps_t[:, ho_s - h0 : ho_s - h0 + nhh, 0:W]
                nc.tensor.matmul(
                    out, lhsT, rhs,
                    start=(n_mm == 0), stop=(n_mm == total - 1),
                )
                return n_mm + 1

            def conv_taps_outer(ot, s, its, blocks, state):
                # tap-outer (khkw order): stationary reuse across blocks
                for it in its:
                    for dh, dw in taps:
                        for bi in blocks:
                            h0, nh, ps_t = state[bi][:3]
                            state[bi][3] = emit_tap(
                                ot, s, it, dh, dw, h0, nh, ps_t,
                                state[bi][3], state[bi][4],
                            )

            def conv_epilogue(ot, s, h0, nh, ps_t):
                o_t = outp.tile([128, HB, W], BF16, tag="out", name="o_t")
                nc.scalar.activation(
                    o_t[:, :nh, :], ps_t[:, :nh, :], AF.Silu,
                    bias=bnb_sb[ot][:], scale=bns_sb[ot][:],
                )
                nc.scalar.dma_start(y_d[s, ot, :, h0 : h0 + nh, :], o_t[:, :nh, :])

            def conv_block_taps(ot, s, h0, nh, ps_t, its, n_mm, total):
                for it in its:
                    for dh, dw in taps:
                        n_mm = emit_tap(ot, s, it, dh, dw, h0, nh, ps_t, n_mm, total)
                return n_mm

            def conv_full_block(ot, s, h0, nh):
                ps_t = psc.tile([128, HB, W], F32, tag="ps", name="ps")
                total = block_total(h0, nh)
                n_mm = conv_block_taps(ot, s, h0, nh, ps_t, range(IT), 0, total)
                assert n_mm == total
                conv_epilogue(ot, s, h0, nh, ps_t)

            # ================= emission schedule =================
            warmup(56)

            combine_pe(0, 0, [0])
            combine_eng(nc.vector, 0, 0, [1])

            # phase-A: it0 taps, tap-outer, first NPA blocks of (s0, ot0)
            state = {}
            for bi, (h0, nh) in enumerate(hblocks[:NPA]):
                ps_t = psc.tile([128, HB, W], F32, tag="ps", name="ps")
                state[bi] = [h0, nh, ps_t, 0, block_total(h0, nh)]
            conv_taps_outer(0, 0, [0], list(range(NPA)), state)

            # s1/ot0 it0 on PE fills the phase-A -> phase-B wait; DVE
            # chains run serialized in conv-deadline order
            combine_pe(0, 1, [0])
            dep = combine_eng(nc.vector, 1, 0, [1], dep=cw_r[0, 1, 0])
            dep = combine_eng(nc.vector, 0, 1, [1], dep=dep)
            combine_eng(nc.vector, 1, 1, range(IT), dep=dep)

            # phase-B: close phase-A blocks with it1 taps
            conv_taps_outer(0, 0, [1], list(range(NPA)), state)
            for bi in range(NPA):
                h0, nh, ps_t, n_mm, total = state[bi]
                assert n_mm == total
                conv_epilogue(0, 0, h0, nh, ps_t)

            # remaining s0/ot0 blocks; then s0/ot1 ((s0,ot1,it0) PE combine
            # first), s1/ot0, s1/ot1
            for h0, nh in hblocks[NPA:]:
                conv_full_block(0, 0, h0, nh)
            combine_pe(1, 0, [0])
            for h0, nh in hblocks:
                conv_full_block(1, 0, h0, nh)
            for h0, nh in hblocks:
                conv_full_block(0, 1, h0, nh)
            for h0, nh in hblocks:
                conv_full_block(1, 1, h0, nh)

    nc.compile()
    return nc


def _get_program():
    if "nc" not in _PROGRAM_CACHE:
        _PROGRAM_CACHE["nc"] = _build_program()
    return _PROGRAM_CACHE["nc"]


def kernel(x, routing_w, routing_b, kernel_weights, bn_gamma, bn_beta, bn_mean, bn_var,
           _trace=False, _trace_kwargs=None):
    x = np.asarray(x, dtype=np.float32)
    routing_w = np.asarray(routing_w, dtype=np.float32)
    routing_b = np.asarray(routing_b, dtype=np.float32)
    kernel_weights = np.asarray(kernel_weights, dtype=np.float32)
    bn_gamma = np.asarray(bn_gamma, dtype=np.float32)
    bn_beta = np.asarray(bn_beta, dtype=np.float32)
    bn_mean = np.asarray(bn_mean, dtype=np.float32)
    bn_var = np.asarray(bn_var, dtype=np.float32)

    bf16 = ml_dtypes.bfloat16
    # host routing: r = sigmoid(GAP(x) @ routing_w.T + routing_b)  (16x8)
    pooled = x.mean(axis=(2, 3))
    logits = pooled @ routing_w.T + routing_b
    r = 1.0 / (1.0 + np.exp(-logits))  # (B, E) fp64 -> fp32
    r = r.astype(np.float32)
    eye = np.eye(128, dtype=np.float32)
    # diag[s, :, e*128:(e+1)*128] = r[s, e] * I
    diag_all = (r[:, :, None, None] * eye[None, None]).transpose(0, 2, 1, 3)
    diag_host_all = np.ascontiguousarray(diag_all).reshape(B, 128, E * 128).astype(bf16)
    rbc_host_all = np.broadcast_to(r[:, None, :], (B, 128, E)).astype(np.float32)

    # wt[ot, it, pair, i, half*SLAB + khkw*128 + o_in]
    # from kernel_weights[e, o, i, kh, kw]; experts packed in pairs
    kw7 = kernel_weights.reshape(E, OT, 128, IT, 128, KS, KS)
    wt_e = np.ascontiguousarray(kw7.transpose(1, 3, 0, 4, 5, 6, 2)).reshape(
        OT, IT, E, 128, SLAB
    )
    wt_host = np.ascontiguousarray(
        wt_e.reshape(OT, IT, E // 2, 2, 128, SLAB).transpose(0, 1, 2, 4, 3, 5)
    ).reshape(OT, IT, E // 2, 128, 2 * SLAB).astype(bf16)
    inv = bn_gamma / np.sqrt(bn_var + BN_EPS)
    bns_host = np.ascontiguousarray(inv).reshape(OT, 128, 1)
    bnb_host = np.ascontiguousarray(bn_beta - bn_mean * inv).reshape(OT, 128, 1)

    x_pad = np.zeros((B, CIN, H, WP), dtype=np.float32)
    x_pad[:, :, :, 1 : 1 + W] = x
    x_pad = x_pad.astype(bf16)
    in_maps = []
    for g in range(NCORES):
        xg = np.ascontiguousarray(
            x_pad[g * SPC : (g + 1) * SPC].reshape(SPC, IT, 128, H, WP)
        )
        in_maps.append(
            {
                "x": xg,
                "wt": wt_host,
                "diag": np.ascontiguousarray(diag_host_all[g * SPC : (g + 1) * SPC]),
                "rbc": np.ascontiguousarray(rbc_host_all[g * SPC : (g + 1) * SPC]),
                "bns": bns_host,
                "bnb": bnb_host,
            }
        )

    nc = _get_program()
    res = run_bass_kernel_spmd(
        nc, in_maps, core_ids=list(range(NCORES)),
        trace=_trace, **(_trace_kwargs or {}),
    )
    _PROGRAM_CACHE["last_result"] = res

    out = np.empty((B, COUT, H, W), dtype=np.float32)
    for g in range(NCORES):
        yg = res.results[g]["y"].astype(np.float32)
        out[g * SPC : (g + 1) * SPC] = yg.reshape(SPC, COUT, H, W)
    return out
